# revision 106
# baseline (speedup 1.0000x reference)
"""Fused multi-head attention block (qkv proj + RMSNorm(q,k) + softmax(QK^T)V
+ out proj), tensor-parallel over 8 TRN2 NeuronCores (2 heads per core).

Per-core layout (matmul operands bf16, PSUM accumulation fp32):
  - q,k computed dim-major (qnT/knT [128, N]) with W stationary.
  - v computed token-major directly (x chunk stationary, Wv moving) into
    vaug [128keys, block, 65] whose col 64 is a ones column: row 64 of the
    PV accumulator is then the softmax denominator. No PE transposes.
  - RMSNorm: one ones-matmul computes q and k sum-of-squares together
    ([2, 1024] out), one Rsqrt ACT op, per-head broadcast back via
    select-matmuls, fused scale multiply on DVE. The squares run on the
    otherwise-idle GPSIMD engine.
  - scores sT [key, tok] per 2-key-block pair; Exp on ACT ([128,1024]
    chunks, bf16 out); PV contracts keys on partitions. No max-subtraction
    is needed: post-RMSNorm |q.k|/sqrt(hd) <= sqrt(hd)*max_scale^2.
  - softmax normalize: per-head 1/den broadcast with a single [2,128]
    select-matmul per query tile; the out-proj matmuls of each query tile
    are interleaved into the next query tile's attention stream so the PE
    never waits on the PSUM->SBUF drains.
  - per-core partial outputs are written bf16 and summed on the host (the
    TP all-reduce) together with bout and the exact v-bias fold (attention
    rows sum to 1, so attn @ (v+bv) = attn@v + bv).
"""

import numpy as np

B, S, D, H = 2, 2048, 1024, 16
HD = D // H            # 64
N = B * S              # 4096 tokens
NCORES = 8
HPC = H // NCORES      # 2 heads per core
PD = HPC * HD          # 128 per-core head dims
EPS = 1e-6
TOK_T = 512            # token tile (free dim)
KB = 128               # key block (partition dim in PV)
VW = HD + 1            # vaug width (64 v dims + ones col)
NKC = D // 128         # 8 contraction chunks

_last_results = None   # test.py introspection (exec_time_ns, profile)
_nc_cache = None


def _build_program():
    global _nc_cache
    if _nc_cache is not None:
        return _nc_cache
    _nc_cache = _build_program_uncached()
    return _nc_cache


def _build_program_uncached():
    import concourse.bacc as bacc
    import concourse.bass as bass
    import concourse.mybir as mybir
    import concourse.tile as tile

    f32 = mybir.dt.float32
    bf16 = mybir.dt.bfloat16
    AF = mybir.ActivationFunctionType
    ALU = mybir.AluOpType

    nc = bacc.Bacc(None, target_bir_lowering=False, debug=False)

    xT_h = nc.declare_dram_parameter("xT", [D, N], bf16, isOutput=False)
    Wq_h = nc.declare_dram_parameter("Wq", [128, NKC, 2 * PD], bf16,
                                     isOutput=False)
    Wv_h = nc.declare_dram_parameter("Wv", [128, NKC, PD], bf16,
                                     isOutput=False)
    Wo_h = nc.declare_dram_parameter("Wo", [PD, D], bf16, isOutput=False)
    bq_h = nc.declare_dram_parameter("bq", [PD, 2], f32, isOutput=False)
    sel_h = nc.declare_dram_parameter("sel", [2, 3, 128], bf16, isOutput=False)
    out_h = nc.declare_dram_parameter("outp", [N, D], bf16, isOutput=True)

    n_tt = N // TOK_T           # 8 token tiles
    n_kb = S // KB              # 16 key blocks per batch
    n_qt = S // TOK_T           # 4 query tiles per batch

    with nc.allow_low_precision(reason="bf16 operands"), \
            tile.TileContext(nc) as tc:
        with (
            tc.tile_pool(name="big", bufs=1) as big,
            tc.tile_pool(name="consts", bufs=1) as consts,
        ):
            # ---- persistent SBUF tensors ----
            qnT = big.tile([PD, N], bf16, tag="qnT")
            knT = big.tile([PD, N], bf16, tag="knT")
            onT = big.tile([PD, N], bf16, tag="onT")
            vaug = big.tile([KB, B * HPC * n_kb, VW], bf16, tag="vaug")
            xts = big.tile([128, NKC, N], bf16, tag="xts")
            Wsb = big.tile([128, NKC, 2 * PD], bf16, tag="Wsb")
            WvSb = big.tile([128, NKC, PD], bf16, tag="WvSb")
            WoSb = big.tile([PD, D], bf16, tag="WoSb")
            bqSb = consts.tile([PD, 2], f32, tag="bqSb")

            # startup-critical order: first qk matmul needs Wsb halves and
            # the first half of xt tile 0; everything else can trail.
            xin = xT_h[:, :].rearrange("(kc p) n -> p kc n", p=128)
            nc.sync.dma_start(out=Wsb[:, 0:NKC // 2, :],
                              in_=Wq_h[:, 0:NKC // 2, :])
            nc.sync.dma_start(out=xts[:, 0:NKC // 2, 0:TOK_T],
                              in_=xin[:, 0:NKC // 2, 0:TOK_T])
            nc.sync.dma_start(out=Wsb[:, NKC // 2:NKC, :],
                              in_=Wq_h[:, NKC // 2:NKC, :])
            nc.sync.dma_start(out=xts[:, NKC // 2:NKC, 0:TOK_T],
                              in_=xin[:, NKC // 2:NKC, 0:TOK_T])
            nc.sync.dma_start(out=WvSb, in_=Wv_h[:, :, :])
            nc.sync.dma_start(out=bqSb, in_=bq_h[:, :])

            # ---- constants ----
            # sel[m] rows 0/1 select head 0 / head 1 partitions on broadcast,
            # premultiplied by the q/k RMSNorm scales (host-provided:
            # multi-partition writes can't start at row 1).
            selA = consts.tile([2, 3, 128], bf16, tag="selA")
            nc.sync.dma_start(out=selA, in_=sel_h[:, :, :])
            # ones2 cols 0/1 sum head 0 / head 1 partitions.
            ones2 = consts.tile([128, 2], bf16, tag="ones2")
            nc.vector.memset(ones2, 0.0)
            nc.vector.memset(ones2[0:HD, 0:1], 1.0)
            nc.vector.memset(ones2[HD:128, 1:2], 1.0)
            epsP = consts.tile([2, 1], f32, tag="epsP")
            nc.vector.memset(epsP, EPS)
            zb = consts.tile([128, 1], f32, tag="zb")
            nc.vector.memset(zb, 0.0)
            # ones column of vaug (col 64); v dims go into cols 0:64.
            nc.vector.memset(vaug[:, :, HD:VW], 1.0)

            def load_xt(t):
                tsl = slice(t * TOK_T, (t + 1) * TOK_T)
                nc.sync.dma_start(out=xts[:, :, tsl], in_=xin[:, :, tsl])

            for t in range(1, 5):
                load_xt(t)
            nc.sync.dma_start(out=WoSb, in_=Wo_h[:, :])

            with (
                tc.tile_pool(name="p1r", bufs=5) as p1r,
                tc.tile_pool(name="p1q", bufs=3) as p1q,
                tc.tile_pool(name="p1s", bufs=2) as p1s,
                tc.tile_pool(name="p2t", bufs=11) as p2t,
                tc.tile_pool(name="p2r", bufs=2) as p2r,
                tc.tile_pool(name="p2b", bufs=4) as p2b,
                tc.tile_pool(name="p3o", bufs=8) as p3o,
                tc.tile_pool(name="psA", bufs=2, space=bass.MemorySpace.PSUM) as psA,
                tc.tile_pool(name="psB", bufs=2, space=bass.MemorySpace.PSUM) as psB,
            ):
                # ============ Phase 1: q/k dim-major + RMSNorm; v natural ===
                # The RMSNorm matmuls of tile t are emitted during tile t+1
                # so the PE never waits on the DVE/Pool/ACT chain. v for the
                # second batch is deferred into phase 2 (the PE has slack
                # there while the ACT streams exps).

                def emit_vna(t, blocks=None):
                    # v token-major: x chunk stationary, Wv moving; the two
                    # heads' 64-dim halves land in adjacent free columns.
                    nblk = TOK_T // KB
                    blocks = range(nblk) if blocks is None else blocks
                    vna = psB.tile([128, TOK_T], f32, tag="B", name="vna")
                    vnav = vna[:, :].rearrange("p (tb vd) -> p tb vd", tb=4)
                    for tb in blocks:
                        t0 = t * TOK_T + tb * KB
                        for kc in range(NKC):
                            nc.tensor.matmul(
                                vnav[:, tb, :],
                                xts[:, kc, t0:t0 + KB],
                                WvSb[:, kc, :],
                                start=(kc == 0), stop=(kc == NKC - 1),
                            )
                    b = t * TOK_T // S
                    blk0 = (b * HPC) * n_kb + (t * TOK_T - b * S) // KB
                    lo, hi = min(blocks), max(blocks) + 1
                    nc.vector.tensor_copy(
                        vaug[:, blk0 + lo:blk0 + hi, 0:HD],
                        vnav[:, lo:hi, 0:HD])
                    nc.vector.tensor_copy(
                        vaug[:, blk0 + n_kb + lo:blk0 + n_kb + hi, 0:HD],
                        vnav[:, lo:hi, HD:PD])

                def phase1_tile(t):
                    tsl = slice(t * TOK_T, (t + 1) * TOK_T)
                    qk = psA.tile([128, 2, TOK_T], f32, tag="bigA", name="qk")
                    for m in range(2):
                        for kc in range(NKC):
                            nc.tensor.matmul(
                                qk[:, m, :],
                                Wsb[:, kc, m * 128:(m + 1) * 128],
                                xts[:, kc, tsl],
                                start=(kc == 0), stop=(kc == NKC - 1),
                            )
                    if t < n_tt // 2:
                        emit_vna(t)
                    # bias-add on DVE (frees the qk PSUM early); squares on
                    # the otherwise-idle GPSIMD engine
                    raw = p1r.tile([128, 2, TOK_T], bf16, tag="raw")
                    for m in range(2):
                        nc.vector.tensor_scalar_add(raw[:, m, :],
                                                    qk[:, m, :],
                                                    bqSb[:, m:m + 1])
                    sq2 = p1q.tile([128, 2, TOK_T], bf16, tag="sq2")
                    for m in range(2):
                        nc.gpsimd.tensor_mul(sq2[:, m, :], raw[:, m, :],
                                             raw[:, m, :])
                    return tsl, raw, sq2

                def phase1_ssum(state):
                    tsl, raw, sq2 = state
                    ssum = psS.tile([2, 2, TOK_T], f32, tag="ssum",
                                    name="ssum")
                    for m in range(2):
                        nc.tensor.matmul(ssum[:, m, :], ones2[:, :],
                                         sq2[:, m, :], start=True, stop=True)
                    return state, ssum

                def phase1_rstd(state1):
                    # rstd = sqrt(HD / ssum + eps) ~= 1/sqrt(ms + eps)
                    state, ssum = state1
                    rms_i = p1s.tile([2, 2, TOK_T], f32, tag="rms_i")
                    nc.vector.reciprocal(rms_i[:, :, :], ssum[:, :, :])
                    rstd = p1s.tile([2, 2, TOK_T], bf16, tag="rstd")
                    nc.scalar.activation(
                        out=rstd[:, :, :], in_=rms_i[:, :, :],
                        func=AF.Sqrt, bias=epsP[:, :], scale=float(HD))
                    return state, rstd

                def phase1_rmsB(stateA):
                    # per-head broadcast (scale folded into sel) + normalize
                    (tsl, raw, sq2), rstd = stateA
                    bcs = p2b.tile([128, 2, TOK_T], f32, tag="bcs2",
                                   name="bcs")
                    for m in range(2):
                        bc = psB.tile([128, TOK_T], f32, tag="B", name="bc")
                        nc.tensor.matmul(bc, selA[:, m, :], rstd[:, m, :],
                                         start=True, stop=True)
                        nc.vector.tensor_copy(bcs[:, m, :], bc)
                    nc.gpsimd.tensor_mul(qnT[:, tsl], raw[:, 0, :],
                                          bcs[:, 0, :])
                    nc.vector.tensor_mul(knT[:, tsl], raw[:, 1, :],
                                         bcs[:, 1, :])

                q0 = []   # emitted tiles awaiting ssum
                q1 = []   # ssum results awaiting rstd
                qb = []   # rstd results awaiting broadcast+normalize
                with tc.tile_pool(name="psS", bufs=1,
                                  space=bass.MemorySpace.PSUM) as psS:
                    for t in range(n_tt):
                        if q1:
                            qb.append(phase1_rstd(q1.pop(0)))
                        if len(qb) >= 2:
                            phase1_rmsB(qb.pop(0))
                        st_t = phase1_tile(t)
                        if q0:
                            q1.append(phase1_ssum(q0.pop(0)))
                        q0.append(st_t)
                        if t == 4:
                            for t2 in range(5, n_tt):
                                load_xt(t2)
                    qb.append(phase1_rstd(q1.pop(0)))
                    q1.append(phase1_ssum(q0.pop(0)))
                    qb.append(phase1_rstd(q1.pop(0)))
                # tiles 5-7 stage B runs during early phase 2 (batch-1 data)

                # ============ Phase 2: attention + interleaved epilogues ====
                # All post-attention work for a query tile (per-head 1/den
                # broadcast, normalize, out-projection) and the deferred b=1
                # v-projections are queued as work items and injected one per
                # key-block pair into the FOLLOWING attention stream, so the
                # PE and DVE chew through them while the ACT streams exps.
                with tc.tile_pool(name="psP", bufs=1,
                                  space=bass.MemorySpace.PSUM) as psP:
                    onesH = consts.tile([1, 128], bf16, tag="onesH")
                    nc.vector.memset(onesH, 1.0)
                    pending = []

                    def head_norm_item(po_h, h, qsl, rec):
                        # bcH = broadcast of 1/den over this head's 64 rows
                        def em():
                            bcH = psB.tile([128, TOK_T], f32, tag="B",
                                           name="bcH")
                            dst = bcH[h * HD:(h + 1) * HD, :]
                            nc.tensor.matmul(dst, onesH[:, 0:HD], rec,
                                             start=True, stop=True)
                            bcs = p2b.tile([64, TOK_T], f32, tag="bc12s",
                                           name="bcs")
                            nc.vector.tensor_copy(bcs, dst)
                            nc.vector.tensor_mul(
                                onT[h * HD:(h + 1) * HD, qsl],
                                po_h[0:HD, :], bcs)
                        return em

                    def outproj_emitters(q0, act_assist=False):
                        ems = []
                        state = {}
                        for tb in range(q0 // 128, (q0 + TOK_T) // 128):
                            for od in range(2):
                                def em(tb=tb, od=od):
                                    if act_assist and od == 1:
                                        psw = psA.tile([128, 2, TOK_T], f32,
                                                       tag="bigA", name="p3a")
                                        ps3 = psw[:, 0, :]
                                    else:
                                        ps3 = psB.tile([128, TOK_T], f32,
                                                       tag="B", name="ps3")
                                    nc.tensor.matmul(
                                        ps3,
                                        onT[:, tb * 128:(tb + 1) * 128],
                                        WoSb[:, od * TOK_T:(od + 1) * TOK_T],
                                        start=True, stop=True)
                                    if od == 0:
                                        state[tb] = p3o.tile([128, D], bf16,
                                                             tag="ot",
                                                             name="ot")
                                    ot = state[tb]
                                    dst = ot[:, od * TOK_T:(od + 1) * TOK_T]
                                    if act_assist and od == 0:
                                        nc.scalar.copy(out=dst, in_=ps3)
                                    else:
                                        nc.vector.tensor_copy(dst, ps3)
                                    if od == 1:
                                        eng = (nc.scalar if act_assist
                                               and tb % 2 else nc.sync)
                                        eng.dma_start(
                                            out=out_h[tb * 128:(tb + 1) * 128,
                                                      0:D],
                                            in_=ot)
                                        del state[tb]
                                ems.append(em)
                        return ems

                    def emit_scores(b, qsl, h, p):
                        hsl = slice(h * HD, (h + 1) * HD)
                        pss = psA.tile([128, 2, TOK_T], f32, tag="bigA",
                                       name="pss")
                        for j in range(2):
                            kb = p * 2 + j
                            k0 = b * S + kb * KB
                            nc.tensor.matmul(
                                pss[:, j, :],
                                knT[hsl, k0:k0 + KB],
                                qnT[hsl, qsl],
                                start=True, stop=True)
                        return pss

                    def finish_pair(w):
                        b, qt, qsl, h, p, pss, po = w
                        pt = p2t.tile([128, 2, TOK_T], bf16, tag="pt")
                        nc.scalar.activation(out=pt, in_=pss, func=AF.Exp,
                                             bias=zb[:, :], scale=1.0)
                        if pending and p != n_kb // 2 - 1:
                            pending.pop(0)()
                        for j in range(2):
                            kb = p * 2 + j
                            nc.tensor.matmul(
                                po[h],
                                vaug[:, (b * HPC + h) * n_kb + kb, :],
                                pt[:, j, :],
                                start=(kb == 0),
                                stop=(kb == n_kb - 1))
                        if p == n_kb // 2 - 1:
                            # head denominator complete: reciprocal now,
                            # broadcast+normalize via the pending queue
                            rec = p2r.tile([1, TOK_T], bf16, tag=f"rec{h}",
                                           name="rec")
                            nc.vector.reciprocal(rec, po[h][HD:VW, :])
                            pending.append(
                                head_norm_item(po[h], h, qsl, rec))
                            if h == HPC - 1:
                                last = (qsl.start == (B - 1) * S
                                        + (n_qt - 1) * TOK_T)
                                pending.extend(
                                    outproj_emitters(qsl.start,
                                                     act_assist=last))

                    prev = None
                    for b in range(B):
                        for qt in range(n_qt):
                            q0 = b * S + qt * TOK_T
                            qsl = slice(q0, q0 + TOK_T)
                            po = [psP.tile([VW, TOK_T], f32, tag=f"po{h}",
                                           name=f"po{h}")
                                  for h in range(HPC)]
                            if b == 0:
                                if qb:
                                    st = qb.pop(0)
                                    pending.append(
                                        lambda st=st: phase1_rmsB(st))
                                for tb in range(4):
                                    pending.append(
                                        lambda t=4 + qt, tb=tb:
                                        emit_vna(t, [tb]))
                            for h in range(HPC):
                                for p in range(n_kb // 2):
                                    cur = (b, qt, qsl, h, p,
                                           emit_scores(b, qsl, h, p), po)
                                    if prev is not None:
                                        finish_pair(prev)
                                    prev = cur
                    finish_pair(prev)
                    while pending:
                        pending.pop(0)()

    nc.compile()
    return nc


def kernel(x, Wqkv, bqkv, Wout, bout, q_scale, k_scale):
    global _last_results
    from concourse.bass_utils import run_bass_kernel_spmd
    import ml_dtypes

    bf16_np = ml_dtypes.bfloat16

    x = np.asarray(x, dtype=np.float32)
    Wqkv = np.asarray(Wqkv, dtype=np.float32)
    bqkv = np.asarray(bqkv, dtype=np.float32)
    Wout = np.asarray(Wout, dtype=np.float32)
    bout = np.asarray(bout, dtype=np.float32)
    q_scale = np.asarray(q_scale, dtype=np.float32)
    k_scale = np.asarray(k_scale, dtype=np.float32)

    def bf(a):
        return np.ascontiguousarray(a).astype(bf16_np)

    xT = bf(x.reshape(N, D).T)
    qs2 = np.tile(q_scale, HPC) / np.sqrt(HD)
    ks2 = np.tile(k_scale, HPC)
    sel_np = np.zeros((2, 3, 128), dtype=np.float32)
    for p in range(PD):
        j = p // HD
        sel_np[j, 0, p] = qs2[p]
        sel_np[j, 1, p] = ks2[p]
        sel_np[j, 2, p] = 1.0
    sel_np = bf(sel_np)
    in_maps = []
    for c in range(NCORES):
        c0 = c * PD
        Wq_s = np.concatenate(
            [Wqkv[:, c0:c0 + PD], Wqkv[:, D + c0:D + c0 + PD]], axis=1)
        Wq_s = Wq_s.reshape(NKC, 128, 2 * PD).transpose(1, 0, 2)
        Wv_s = Wqkv[:, 2 * D + c0:2 * D + c0 + PD]
        Wv_s = Wv_s.reshape(NKC, 128, PD).transpose(1, 0, 2)
        in_maps.append({
            "xT": xT,
            "Wq": bf(Wq_s),
            "Wv": bf(Wv_s),
            "Wo": bf(Wout[c0:c0 + PD, :]),
            "bq": np.ascontiguousarray(
                np.stack([bqkv[c0:c0 + PD], bqkv[D + c0:D + c0 + PD]],
                         axis=1)),
            "sel": sel_np,
        })

    nc = _build_program()
    res = run_bass_kernel_spmd(nc, in_maps, core_ids=list(range(NCORES)))
    _last_results = res

    acc = np.zeros((N, D), dtype=np.float32)
    for c in range(NCORES):
        acc += np.asarray(res.results[c]["outp"], dtype=np.float32)
    acc = acc + bout + bqkv[2 * D:3 * D] @ Wout
    return acc.reshape(B, S, D).astype(np.float32)
